# revision 21
# baseline (speedup 1.0000x reference)
"""Multi-head self-attention (1x1-conv QKV + attention + 1x1-conv proj) for
Trainium2, distributed over 8 NeuronCores.

Problem (hardcoded shapes):
  x: (4, 256, 64, 64) f32;  DIM=256, heads=8, head_dim=32, S=64*64=4096.

Sharding: 8 cores = 4 batches x 2 query-halves. Every core holds the full
key/value sequence for its batch (no cross-core reduction) and computes
attention + projection for its half of the queries (NQ=2048).

Core ideas of this version (vs the 790us baseline):
  - exp() is split across BOTH the scalar engine (ACT spline exp, the only
    native transcendental) and the vector engine, which computes exp via the
    Schraudolph bit-trick in ONE op/element: i16 = rne(A*scale*s + B)
    bitcast to f16 gives 2^(..) with a ~+-3% mantissa sawtooth that the
    softmax normalization largely cancels (measured end-to-end rel err
    ~6e-3 vs the 2e-2 budget at a ~45% vector share).
  - items are (query-block, head-quad): 4 heads x 256 queries. The scores
    matmuls run 4-way row-tiled (4x 32-row PE tiles concurrently), and the
    P@V matmuls run col-tiled 2 heads at a time with the softmax
    denominator riding as a 33rd ones-column of V.
  - k-bias is dropped (softmax is invariant to per-query score shifts), and
    the v-bias is folded into the projection bias on the host
    (bp' = bp + Wp @ bv, exact because softmax rows sum to 1).
"""

import numpy as np

import concourse.bacc as bacc
import concourse.bass as bass
import concourse.tile as tile
from concourse import mybir

# Pin every activation we use (Exp for softmax, Ln+Exp for 1/den, Copy) to
# the single table set that contains them all. Without this the per-function
# set selector alternates exp_and_others <-> natural_log sets, paying a
# ~1.3us ACT_TABLE_LOAD per switch, twice per item. Only strips functions
# from the python-side view (set ids stay canonical), so walrus is unaffected.
_PINNED_SET = "natural_log_exp_and_others"
_PINNED_FUNCS = {
    mybir.ActivationFunctionType.Exp,
    mybir.ActivationFunctionType.Ln,
    mybir.ActivationFunctionType.Copy,
    mybir.ActivationFunctionType.Identity,
}
_orig_get_tables = bacc.get_activation_tables


def _pinned_get_tables(arch):
    tabs = _orig_get_tables(arch)
    for name, funcs in tabs.items():
        if name != _PINNED_SET:
            tabs[name] = funcs - _PINNED_FUNCS
    return tabs


bacc.get_activation_tables = _pinned_get_tables

C = 256  # channels
NH = 8  # heads
D = 32  # head dim
QB = 256  # queries per item
F32 = mybir.dt.float32
F32R = mybir.dt.float32r
F16 = mybir.dt.float16
I16 = mybir.dt.int16

SCALE = float(D) ** -0.5
A16 = float(2**10) / float(np.log(2))  # f16 Schraudolph scale
B16 = float(15 * 2**10) - 60.0  # f16 exponent bias - sawtooth mean-centering

FRAC_ACT = 0.54  # fraction of exp groups on the scalar engine
RING_G = 36  # et ring size in groups (2 half-items in flight + slack)


def _group_sizes(T):
    # chunks per half-item = 2*T (2 head-streams x T key chunks); exp-call
    # groups of 4 chunks (2 PSUM banks, 1024 elems) so the score PSUM pool
    # can triple-buffer (deeper PE run-ahead).
    n = 2 * T
    sizes = [4] * (n // 4)
    assert sum(sizes) == n
    return sizes


def build_nc(S, NQ, frac_act=FRAC_ACT):
    T = S // 128  # key chunks
    NQB = NQ // QB  # query blocks
    GSIZES = _group_sizes(T)
    NG = len(GSIZES)

    nc = bacc.Bacc("TRN2", target_bir_lowering=False)

    xc_d = nc.dram_tensor("xc", [C, S], F32R, kind="ExternalInput")
    xq_d = nc.dram_tensor("xq", [C, NQ], F32R, kind="ExternalInput")
    wqkvT_d = nc.dram_tensor("wqkvT", [C, 3 * C], F32R, kind="ExternalInput")
    bq_d = nc.dram_tensor("bq", [C, 1], F32, kind="ExternalInput")
    wpT_d = nc.dram_tensor("wpT", [C, C], F16, kind="ExternalInput")
    bp_d = nc.dram_tensor("bp", [C, 1], F32, kind="ExternalInput")
    out_d = nc.dram_tensor("out", [C, NQ], F32, kind="ExternalOutput")

    with tile.TileContext(nc) as tc:
        with (
            tc.tile_pool(name="big", bufs=1) as big,
            tc.tile_pool(name="persist", bufs=1) as per,
            tc.tile_pool(name="r", bufs=4) as r_pool,
            tc.tile_pool(name="R", bufs=4) as R_pool,
            tc.tile_pool(name="po", bufs=4) as po_pool,
        ):
            # ---- persistent SBUF tiles ----
            xc_sb = big.tile([128, 2, S], F32R, tag="big", name="xcsb")
            xq_sb = per.tile([128, 2, NQ], F32R, tag="xq", name="xq")
            w_sb = per.tile([128, 2, 3 * C], F32R, tag="w", name="w")
            wp_sb = per.tile([128, 2, C], F16, tag="wp", name="wp")
            bq_sb = per.tile([128, 2], F32, tag="bq", name="bq")
            bp_sb = per.tile([128, 2], F32, tag="bp", name="bp")
            qt_sb = per.tile([128, 2, NQ], F16, tag="qt", name="qt")
            kt_sb = per.tile([128, 2, S], F16, tag="kt", name="kt")
            v_sb = per.tile([128, T, NH, D + 1], F16, tag="v", name="v")
            ot_sb = per.tile([128, 2, NQ], F16, tag="ot", name="ot")

            for k in range(2):
                psl = slice(128 * k, 128 * (k + 1))
                # xc in 512-col chunks so downstream matmuls can start early
                for n in range(S // 512):
                    nc.sync.dma_start(
                        out=xc_sb[:, k, 512 * n : 512 * (n + 1)],
                        in_=xc_d[psl, 512 * n : 512 * (n + 1)],
                    )
                for n in range(NQ // 512):
                    nc.sync.dma_start(
                        out=xq_sb[:, k, 512 * n : 512 * (n + 1)],
                        in_=xq_d[psl, 512 * n : 512 * (n + 1)],
                    )
                nc.sync.dma_start(out=w_sb[:, k, :], in_=wqkvT_d[psl, :])
                nc.sync.dma_start(out=wp_sb[:, k, :], in_=wpT_d[psl, :])
                nc.sync.dma_start(out=bq_sb[:, k : k + 1], in_=bq_d[psl, :])
                nc.sync.dma_start(out=bp_sb[:, k : k + 1], in_=bp_d[psl, :])
            # ones column for the softmax-denominator trick
            nc.vector.memset(v_sb[:, :, :, D : D + 1], 1.0)

            # ---- QKV projections ----
            with tc.tile_pool(name="ps_setup", bufs=4, space="PSUM") as ps_setup:
                # K^T: (c' on partitions, keys on free), f16, no bias (a
                # per-query score shift is softmax-invariant).
                for n in range(S // 512):
                    for m in range(2):
                        ps = ps_setup.tile([128, 512], F32, tag="mm", name="psk")
                        for k in range(2):
                            nc.tensor.matmul(
                                ps,
                                w_sb[:, k, 256 + 128 * m : 256 + 128 * (m + 1)],
                                xc_sb[:, k, 512 * n : 512 * (n + 1)],
                                start=(k == 0),
                                stop=(k == 1),
                            )
                        # split copies between scalar/vector engines (both
                        # are otherwise idle during setup)
                        dst = kt_sb[:, m, 512 * n : 512 * (n + 1)]
                        if n % 2 == 0:
                            nc.scalar.copy(out=dst, in_=ps)
                        else:
                            nc.vector.tensor_copy(out=dst, in_=ps)
                # Q^T with q-bias folded in
                for n in range(NQ // 512):
                    for m in range(2):
                        ps = ps_setup.tile([128, 512], F32, tag="mm", name="psq")
                        for k in range(2):
                            nc.tensor.matmul(
                                ps,
                                w_sb[:, k, 128 * m : 128 * (m + 1)],
                                xq_sb[:, k, 512 * n : 512 * (n + 1)],
                                start=(k == 0),
                                stop=(k == 1),
                            )
                        nc.vector.tensor_scalar_add(
                            out=qt_sb[:, m, 512 * n : 512 * (n + 1)],
                            in0=ps,
                            scalar1=bq_sb[:, m : m + 1],
                        )
                # V in natural layout (keys on partitions): (128, t, h, d)
                # f16; v-bias folded into the projection bias on the host.
                for t in range(T):
                    ps = ps_setup.tile([128, 256], F32, tag="mm", name="psv")
                    for k in range(2):
                        nc.tensor.matmul(
                            ps,
                            xc_sb[:, k, 128 * t : 128 * (t + 1)],
                            w_sb[:, k, 512:768],
                            start=(k == 0),
                            stop=(k == 1),
                        )
                    nc.vector.tensor_copy(
                        out=v_sb[:, t, :, 0:D],
                        in_=ps.rearrange("p (h d) -> p h d", h=NH),
                    )

            # exp ring buffer: chunk c of group g of item i lives at
            # et[:, (i*NG+g) % RING_G, c, :]
            et = big.tile([128, RING_G, GSIZES[0], QB], F16, tag="big", name="et")

            # ---- attention + projection ----
            with (
                tc.tile_pool(name="ps_s", bufs=3, space="PSUM") as ps_s,
                tc.tile_pool(name="ps_u", bufs=2, space="PSUM") as ps_u,
            ):
                # half-items: (qb, gp, hp) = a head PAIR (heads 4*gp+2*hp,
                # 4*gp+2*hp+1) over one query block. chunk c = 2*t + i'.
                halves = [
                    (qb, gp, hp)
                    for qb in range(NQB)
                    for gp in range(2)
                    for hp in range(2)
                ]
                gstart = np.cumsum([0] + GSIZES)
                # chunk-position -> PSUM slot permutation: adjacent chunks
                # execute concurrently (2 row-tiled streams) and must write
                # different PSUM banks (2 slots/bank). P6 gives bank order
                # 0,1,2,0,1,2 across positions.
                PERM = {6: [0, 2, 4, 1, 3, 5], 4: [0, 2, 1, 3]}

                def emit_scores_group(hidx, qb, gp, hp, g):
                    gsz = GSIZES[g]
                    qsl = slice(QB * qb, QB * (qb + 1))
                    ps = ps_s.tile([128, gsz, QB], F32, tag="s", name="pss")
                    for p in range(gsz):
                        c = gstart[g] + p
                        t, ii = divmod(c, 2)
                        i = 2 * hp + ii
                        nc.tensor.matmul(
                            ps[:, PERM[gsz][p], :],
                            kt_sb[32 * i : 32 * (i + 1), gp, 128 * t : 128 * (t + 1)],
                            qt_sb[32 * i : 32 * (i + 1), gp, qsl],
                            start=True,
                            stop=True,
                            tile_position=(32 * i, 0),
                        )
                    eslot = et[:, (hidx * NG + g) % RING_G, 0:gsz, :]
                    return ps, eslot

                act_acc = [0.0]

                def emit_exp(ps, eslot, gsz):
                    act_acc[0] += frac_act
                    if act_acc[0] >= 1.0:
                        act_acc[0] -= 1.0
                        nc.scalar.activation(
                            out=eslot,
                            in_=ps,
                            func=mybir.ActivationFunctionType.Exp,
                            scale=SCALE,
                        )
                    else:
                        nc.vector.tensor_scalar(
                            out=eslot.bitcast(I16),
                            in0=ps,
                            scalar1=A16 * SCALE,
                            scalar2=B16,
                            op0=mybir.AluOpType.mult,
                            op1=mybir.AluOpType.add,
                        )

                def emit_pv(pending, lo, hi):
                    hidx, qb, gp, hp, uo = pending
                    for c in range(lo, hi):
                        t, ii = divmod(c, 2)
                        g = int(np.searchsorted(gstart, c, side="right")) - 1
                        s = PERM[GSIZES[g]][c - gstart[g]]
                        cp = 64 * ii
                        nc.tensor.matmul(
                            uo[cp : cp + D + 1, hp, :],
                            v_sb[:, t, 4 * gp + 2 * hp + ii, :],
                            et[:, (hidx * NG + g) % RING_G, s, :],
                            start=(t == 0),
                            stop=(t == T - 1),
                            tile_position=(0, cp),
                        )

                def emit_normalize(done):
                    # normalize all 4 heads of a finished (qb, gp) pair.
                    # dens live at uo partitions 32 (ii=0) / 96 (ii=1) x
                    # free-slot hp. 1/den runs on the scalar engine as
                    # exp(-ln(den)) -- ln+exp share one ACT table set, and
                    # the DVE's RECIPROCAL instruction is ~8 cycles/elem.
                    qb, gp, uo = done
                    qsl = slice(QB * qb, QB * (qb + 1))
                    u = r_pool.tile([1, 4, QB], F32, tag="r", name="lden")
                    rr = r_pool.tile([1, 4, QB], F32, tag="r", name="rden")
                    nc.scalar.activation(
                        out=u[:, 0:2, :],
                        in_=uo[32:33, :, :],
                        func=mybir.ActivationFunctionType.Ln,
                    )
                    nc.scalar.activation(
                        out=u[:, 2:4, :],
                        in_=uo[96:97, :, :],
                        func=mybir.ActivationFunctionType.Ln,
                    )
                    nc.scalar.activation(
                        out=rr,
                        in_=u,
                        func=mybir.ActivationFunctionType.Exp,
                        scale=-1.0,
                    )
                    R_a = R_pool.tile([D, 2, QB], F32, tag="R", name="Ra")
                    R_b = R_pool.tile([D, 2, QB], F32, tag="R", name="Rb")
                    nc.gpsimd.partition_broadcast(R_a, rr[:, 0:2, :])
                    nc.gpsimd.partition_broadcast(R_b, rr[:, 2:4, :])
                    for j in range(2):  # hp slot
                        for ii in range(2):
                            h4 = 2 * j + ii  # head % 4
                            Rt = R_a if ii == 0 else R_b
                            nc.vector.tensor_mul(
                                out=ot_sb[32 * h4 : 32 * h4 + D, gp, qsl],
                                in0=uo[64 * ii : 64 * ii + D, j, :],
                                in1=Rt[:, j, :],
                            )

                def emit_proj(qb):
                    # borrows a ps_s buffer (1 of its 3 banks); both m-halves
                    # run sequentially inside one allocation so the uo pool
                    # rotation stays strictly uo-only.
                    qsl = slice(QB * qb, QB * (qb + 1))
                    pp = ps_s.tile([128, 2, QB], F32, tag="s", name="psp")
                    for m in range(2):
                        for k in range(2):
                            nc.tensor.matmul(
                                pp[:, m, :],
                                wp_sb[:, k, 128 * m : 128 * (m + 1)],
                                ot_sb[:, k, qsl],
                                start=(k == 0),
                                stop=(k == 1),
                            )
                        po = po_pool.tile([128, QB], F32, tag="po", name="po")
                        nc.vector.tensor_scalar_add(
                            out=po, in0=pp[:, m, :], scalar1=bp_sb[:, m : m + 1]
                        )
                        nc.sync.dma_start(
                            out=out_d[128 * m : 128 * (m + 1), qsl], in_=po
                        )

                pending = None
                uo = None
                to_norm = None  # finished pair awaiting normalize
                to_proj = None  # qb awaiting projection
                for hidx, (qb, gp, hp) in enumerate(halves):
                    if hp == 0:
                        uo = ps_u.tile([128, 2, QB], F32, tag="u", name="psu")
                    pv_lo = 0
                    for g in range(NG):
                        ps, eslot = emit_scores_group(hidx, qb, gp, hp, g)
                        emit_exp(ps, eslot, GSIZES[g])
                        if pending is not None:
                            pv_hi = (2 * T) * (g + 1) // NG
                            emit_pv(pending, pv_lo, pv_hi)
                            pv_lo = pv_hi
                        # deferred retire work from the previous pair, placed
                        # mid-body so the engine FIFOs always have ready work
                        # queued ahead of the normalize/proj dependency chain
                        if g == 1 and to_norm is not None:
                            emit_normalize(to_norm)
                            if to_norm[1] == 1:  # gp == 1: query block done
                                to_proj = to_norm[0]
                            to_norm = None
                        if g == 6 and to_proj is not None:
                            emit_proj(to_proj)
                            to_proj = None
                    if pending is not None:
                        emit_pv(pending, pv_lo, 2 * T)
                        if pending[3] == 1:  # finished an (qb, gp) pair
                            to_norm = (pending[1], pending[2], pending[4])
                    pending = (hidx, qb, gp, hp, uo)
                # drain
                emit_pv(pending, 0, 2 * T)
                if to_norm is not None:
                    emit_normalize(to_norm)
                    if to_norm[1] == 1:
                        emit_proj(to_norm[0])
                    to_norm = None
                emit_normalize((pending[1], pending[2], pending[4]))
                emit_proj(pending[1])

    nc.compile()
    return nc


def _make_in_maps(x, w_qkv, b_qkv, w_proj, b_proj, n_cores=8):
    B, Cx, Hi, Wi = x.shape
    S = Hi * Wi
    NQ = S * B // n_cores
    xr = np.ascontiguousarray(x.reshape(B, Cx, S).astype(np.float32))
    wqkvT = np.ascontiguousarray(w_qkv.astype(np.float32).T)
    bq = np.ascontiguousarray(b_qkv[:Cx].astype(np.float32).reshape(Cx, 1))
    bv = b_qkv[2 * Cx : 3 * Cx].astype(np.float32)
    wpT = np.ascontiguousarray(w_proj.astype(np.float32).T.astype(np.float16))
    bp_eff = (b_proj.astype(np.float32) + w_proj.astype(np.float32) @ bv).reshape(
        Cx, 1
    )
    bp_eff = np.ascontiguousarray(bp_eff)
    halves = n_cores // B
    in_maps = []
    for core in range(n_cores):
        b, half = divmod(core, halves)
        xq = np.ascontiguousarray(xr[b][:, half * NQ : (half + 1) * NQ])
        in_maps.append(
            {"xc": xr[b], "xq": xq, "wqkvT": wqkvT, "bq": bq, "wpT": wpT, "bp": bp_eff}
        )
    return in_maps, (B, Cx, Hi, Wi, S, NQ)


_NC_CACHE = {}


def run(x, w_qkv, b_qkv, w_proj, b_proj, trace=False, **spmd_kwargs):
    from concourse.bass_utils import run_bass_kernel_spmd

    in_maps, (B, Cx, Hi, Wi, S, NQ) = _make_in_maps(x, w_qkv, b_qkv, w_proj, b_proj)
    key = (S, NQ)
    if key not in _NC_CACHE:
        _NC_CACHE[key] = build_nc(S, NQ)
    nc = _NC_CACHE[key]
    res = run_bass_kernel_spmd(
        nc, in_maps, core_ids=list(range(8)), trace=trace, **spmd_kwargs
    )
    outs = [r["out"] for r in res.results]
    halves = 8 // B
    full = np.empty((B, Cx, S), np.float32)
    for b in range(B):
        for half in range(halves):
            full[b, :, half * NQ : (half + 1) * NQ] = outs[halves * b + half]
    return full.reshape(B, Cx, Hi, Wi), res


def kernel(x, w_qkv, b_qkv, w_proj, b_proj):
    out, _ = run(x, w_qkv, b_qkv, w_proj, b_proj)
    return out


# revision 23
# speedup vs baseline: 1.1234x; 1.1234x over previous
"""Multi-head self-attention (1x1-conv QKV + attention + 1x1-conv proj) for
Trainium2, distributed over 8 NeuronCores.

Problem (hardcoded shapes):
  x: (4, 256, 64, 64) f32;  DIM=256, heads=8, head_dim=32, S=64*64=4096.

Sharding: 8 cores = 4 batches x 2 query-halves. Every core holds the full
key/value sequence for its batch (no cross-core reduction) and computes
attention + projection for its half of the queries (NQ=2048).

Core ideas of this version (vs the 790us baseline):
  - exp() is split across BOTH the scalar engine (ACT spline exp, the only
    native transcendental) and the vector engine, which computes exp via the
    Schraudolph bit-trick in ONE op/element: i16 = rne(A*scale*s + B)
    bitcast to f16 gives 2^(..) with a ~+-3% mantissa sawtooth that the
    softmax normalization largely cancels (measured end-to-end rel err
    ~6e-3 vs the 2e-2 budget at a ~45% vector share).
  - items are (query-block, head-quad): 4 heads x 256 queries. The scores
    matmuls run 4-way row-tiled (4x 32-row PE tiles concurrently), and the
    P@V matmuls run col-tiled 2 heads at a time with the softmax
    denominator riding as a 33rd ones-column of V.
  - k-bias is dropped (softmax is invariant to per-query score shifts), and
    the v-bias is folded into the projection bias on the host
    (bp' = bp + Wp @ bv, exact because softmax rows sum to 1).
"""

import numpy as np

import concourse.bacc as bacc
import concourse.bass as bass
import concourse.tile as tile
from concourse import mybir

# Pin every activation we use (Exp for softmax, Ln+Exp for 1/den, Copy) to
# the single table set that contains them all. Without this the per-function
# set selector alternates exp_and_others <-> natural_log sets, paying a
# ~1.3us ACT_TABLE_LOAD per switch, twice per item. Only strips functions
# from the python-side view (set ids stay canonical), so walrus is unaffected.
_PINNED_SET = "natural_log_exp_and_others"
_PINNED_FUNCS = {
    mybir.ActivationFunctionType.Exp,
    mybir.ActivationFunctionType.Ln,
    mybir.ActivationFunctionType.Copy,
    mybir.ActivationFunctionType.Identity,
}
_orig_get_tables = bacc.get_activation_tables


def _pinned_get_tables(arch):
    tabs = _orig_get_tables(arch)
    for name, funcs in tabs.items():
        if name != _PINNED_SET:
            tabs[name] = funcs - _PINNED_FUNCS
    return tabs


bacc.get_activation_tables = _pinned_get_tables

C = 256  # channels
NH = 8  # heads
D = 32  # head dim
QB = 256  # queries per item
F32 = mybir.dt.float32
F32R = mybir.dt.float32r
F16 = mybir.dt.float16
I16 = mybir.dt.int16

SCALE = float(D) ** -0.5
A16 = float(2**10) / float(np.log(2))  # f16 Schraudolph scale
B16 = float(15 * 2**10) - 60.0  # f16 exponent bias - sawtooth mean-centering

FRAC_ACT = 0.53  # fraction of exp groups on the scalar engine
RING_G = 26  # et ring size in groups (22 groups/item + slack)


def _group_sizes(T):
    # chunks per half-item = 2*T (2 head-streams x T key chunks); exp-call
    # groups of 6 chunks (3 PSUM banks, 1536 elems) with a 4-chunk tail.
    n = 2 * T
    sizes = [6] * ((n - 4) // 6) + [4]
    assert sum(sizes) == n
    return sizes


def build_nc(S, NQ, frac_act=FRAC_ACT):
    T = S // 128  # key chunks
    NQB = NQ // QB  # query blocks
    GSIZES = _group_sizes(T)
    NG = len(GSIZES)

    nc = bacc.Bacc("TRN2", target_bir_lowering=False)

    xc_d = nc.dram_tensor("xc", [C, S], F32R, kind="ExternalInput")
    xq_d = nc.dram_tensor("xq", [C, NQ], F32R, kind="ExternalInput")
    wqkvT_d = nc.dram_tensor("wqkvT", [C, 3 * C], F32R, kind="ExternalInput")
    bq_d = nc.dram_tensor("bq", [C, 1], F32, kind="ExternalInput")
    wpT_d = nc.dram_tensor("wpT", [C, C], F16, kind="ExternalInput")
    bp_d = nc.dram_tensor("bp", [C, 1], F32, kind="ExternalInput")
    out_d = nc.dram_tensor("out", [C, NQ], F32, kind="ExternalOutput")

    with tile.TileContext(nc) as tc:
        with (
            tc.tile_pool(name="big", bufs=1) as big,
            tc.tile_pool(name="persist", bufs=1) as per,
            tc.tile_pool(name="r", bufs=4) as r_pool,
            tc.tile_pool(name="R", bufs=4) as R_pool,
            tc.tile_pool(name="po", bufs=4) as po_pool,
        ):
            # ---- persistent SBUF tiles ----
            xc_sb = big.tile([128, 2, S], F32R, tag="big", name="xcsb")
            xq_sb = per.tile([128, 2, NQ], F32R, tag="xq", name="xq")
            w_sb = per.tile([128, 2, 3 * C], F32R, tag="w", name="w")
            wp_sb = per.tile([128, 2, C], F16, tag="wp", name="wp")
            bq_sb = per.tile([128, 2], F32, tag="bq", name="bq")
            bp_sb = per.tile([128, 2], F32, tag="bp", name="bp")
            qt_sb = per.tile([128, 2, NQ], F16, tag="qt", name="qt")
            kt_sb = per.tile([128, 2, S], F16, tag="kt", name="kt")
            v_sb = per.tile([128, T, NH, D + 1], F16, tag="v", name="v")
            ot_sb = per.tile([128, 2, NQ], F16, tag="ot", name="ot")

            # small tensors first: the first QKV matmul needs w_sb, which
            # must not queue behind 4MB of xc chunks
            for k in range(2):
                psl = slice(128 * k, 128 * (k + 1))
                nc.sync.dma_start(out=w_sb[:, k, :], in_=wqkvT_d[psl, :])
                nc.sync.dma_start(out=wp_sb[:, k, :], in_=wpT_d[psl, :])
                nc.sync.dma_start(out=bq_sb[:, k : k + 1], in_=bq_d[psl, :])
                nc.sync.dma_start(out=bp_sb[:, k : k + 1], in_=bp_d[psl, :])
            for k in range(2):
                psl = slice(128 * k, 128 * (k + 1))
                # xc in 512-col chunks so downstream matmuls can start early
                for n in range(S // 512):
                    nc.sync.dma_start(
                        out=xc_sb[:, k, 512 * n : 512 * (n + 1)],
                        in_=xc_d[psl, 512 * n : 512 * (n + 1)],
                    )
                for n in range(NQ // 512):
                    nc.sync.dma_start(
                        out=xq_sb[:, k, 512 * n : 512 * (n + 1)],
                        in_=xq_d[psl, 512 * n : 512 * (n + 1)],
                    )
            # ones column for the softmax-denominator trick
            nc.vector.memset(v_sb[:, :, :, D : D + 1], 1.0)

            # ---- QKV projections ----
            with tc.tile_pool(name="ps_setup", bufs=4, space="PSUM") as ps_setup:
                # K^T: (c' on partitions, keys on free), f16, no bias (a
                # per-query score shift is softmax-invariant).
                for n in range(S // 512):
                    for m in range(2):
                        ps = ps_setup.tile([128, 512], F32, tag="mm", name="psk")
                        for k in range(2):
                            nc.tensor.matmul(
                                ps,
                                w_sb[:, k, 256 + 128 * m : 256 + 128 * (m + 1)],
                                xc_sb[:, k, 512 * n : 512 * (n + 1)],
                                start=(k == 0),
                                stop=(k == 1),
                            )
                        # split copies between scalar/vector engines (both
                        # are otherwise idle during setup)
                        dst = kt_sb[:, m, 512 * n : 512 * (n + 1)]
                        if n % 2 == 0:
                            nc.scalar.copy(out=dst, in_=ps)
                        else:
                            nc.vector.tensor_copy(out=dst, in_=ps)
                # Q^T with q-bias folded in
                for n in range(NQ // 512):
                    for m in range(2):
                        ps = ps_setup.tile([128, 512], F32, tag="mm", name="psq")
                        for k in range(2):
                            nc.tensor.matmul(
                                ps,
                                w_sb[:, k, 128 * m : 128 * (m + 1)],
                                xq_sb[:, k, 512 * n : 512 * (n + 1)],
                                start=(k == 0),
                                stop=(k == 1),
                            )
                        nc.vector.tensor_scalar_add(
                            out=qt_sb[:, m, 512 * n : 512 * (n + 1)],
                            in0=ps,
                            scalar1=bq_sb[:, m : m + 1],
                        )
                # V in natural layout (keys on partitions): (128, t, h, d)
                # f16; v-bias folded into the projection bias on the host.
                for t in range(T):
                    ps = ps_setup.tile([128, 256], F32, tag="mm", name="psv")
                    for k in range(2):
                        nc.tensor.matmul(
                            ps,
                            xc_sb[:, k, 128 * t : 128 * (t + 1)],
                            w_sb[:, k, 512:768],
                            start=(k == 0),
                            stop=(k == 1),
                        )
                    nc.vector.tensor_copy(
                        out=v_sb[:, t, :, 0:D],
                        in_=ps.rearrange("p (h d) -> p h d", h=NH),
                    )

            # exp ring buffer: chunk c of group g of item i lives at
            # et[:, (i*NG+g) % RING_G, c, :]
            et = big.tile([128, RING_G, 6, QB], F16, tag="big", name="et")

            # ---- attention + projection ----
            with (
                tc.tile_pool(name="ps_s", bufs=2, space="PSUM") as ps_s,
                tc.tile_pool(name="ps_u", bufs=2, space="PSUM") as ps_u,
            ):
                # half-items: (qb, gp, hp) = a head PAIR (heads 4*gp+2*hp,
                # 4*gp+2*hp+1) over one query block. chunk c = 2*t + i'.
                halves = [
                    (qb, gp, hp)
                    for qb in range(NQB)
                    for gp in range(2)
                    for hp in range(2)
                ]
                gstart = np.cumsum([0] + GSIZES)
                # chunk-position -> PSUM slot permutation: adjacent chunks
                # execute concurrently (2 row-tiled streams) and must write
                # different PSUM banks (2 slots/bank). P6 gives bank order
                # 0,1,2,0,1,2 across positions.
                PERM = {6: [0, 2, 4, 1, 3, 5], 4: [0, 2, 1, 3]}

                def emit_scores_group(hidx, qb, gp, hp, g):
                    gsz = GSIZES[g]
                    qsl = slice(QB * qb, QB * (qb + 1))
                    ps = ps_s.tile([128, gsz, QB], F32, tag="s", name="pss")
                    for p in range(gsz):
                        c = gstart[g] + p
                        t, ii = divmod(c, 2)
                        i = 2 * hp + ii
                        nc.tensor.matmul(
                            ps[:, PERM[gsz][p], :],
                            kt_sb[32 * i : 32 * (i + 1), gp, 128 * t : 128 * (t + 1)],
                            qt_sb[32 * i : 32 * (i + 1), gp, qsl],
                            start=True,
                            stop=True,
                            tile_position=(32 * i, 0),
                        )
                    eslot = et[:, (hidx * NG + g) % RING_G, 0:gsz, :]
                    return ps, eslot

                act_acc = [0.0]

                def emit_exp(ps, eslot, gsz):
                    act_acc[0] += frac_act
                    if act_acc[0] >= 1.0:
                        act_acc[0] -= 1.0
                        nc.scalar.activation(
                            out=eslot,
                            in_=ps,
                            func=mybir.ActivationFunctionType.Exp,
                            scale=SCALE,
                        )
                    else:
                        nc.vector.tensor_scalar(
                            out=eslot.bitcast(I16),
                            in0=ps,
                            scalar1=A16 * SCALE,
                            scalar2=B16,
                            op0=mybir.AluOpType.mult,
                            op1=mybir.AluOpType.add,
                        )

                def emit_pv(pending, lo, hi):
                    hidx, qb, gp, hp, uo = pending
                    for c in range(lo, hi):
                        t, ii = divmod(c, 2)
                        g = int(np.searchsorted(gstart, c, side="right")) - 1
                        s = PERM[GSIZES[g]][c - gstart[g]]
                        cp = 64 * ii
                        nc.tensor.matmul(
                            uo[cp : cp + D + 1, hp, :],
                            v_sb[:, t, 4 * gp + 2 * hp + ii, :],
                            et[:, (hidx * NG + g) % RING_G, s, :],
                            start=(t == 0),
                            stop=(t == T - 1),
                            tile_position=(0, cp),
                        )

                def emit_normalize(done):
                    # normalize all 4 heads of a finished (qb, gp) pair.
                    # dens live at uo partitions 32 (ii=0) / 96 (ii=1) x
                    # free-slot hp. 1/den runs on the scalar engine as
                    # exp(-ln(den)) -- ln+exp share one ACT table set, and
                    # the DVE's RECIPROCAL instruction is ~8 cycles/elem.
                    qb, gp, uo = done
                    qsl = slice(QB * qb, QB * (qb + 1))
                    u = r_pool.tile([1, 4, QB], F32, tag="r", name="lden")
                    rr = r_pool.tile([1, 4, QB], F32, tag="r", name="rden")
                    nc.scalar.activation(
                        out=u[:, 0:2, :],
                        in_=uo[32:33, :, :],
                        func=mybir.ActivationFunctionType.Ln,
                    )
                    nc.scalar.activation(
                        out=u[:, 2:4, :],
                        in_=uo[96:97, :, :],
                        func=mybir.ActivationFunctionType.Ln,
                    )
                    nc.scalar.activation(
                        out=rr,
                        in_=u,
                        func=mybir.ActivationFunctionType.Exp,
                        scale=-1.0,
                    )
                    R_a = R_pool.tile([D, 2, QB], F32, tag="R", name="Ra")
                    R_b = R_pool.tile([D, 2, QB], F32, tag="R", name="Rb")
                    nc.gpsimd.partition_broadcast(R_a, rr[:, 0:2, :])
                    nc.gpsimd.partition_broadcast(R_b, rr[:, 2:4, :])
                    for j in range(2):  # hp slot
                        for ii in range(2):
                            h4 = 2 * j + ii  # head % 4
                            Rt = R_a if ii == 0 else R_b
                            nc.vector.tensor_mul(
                                out=ot_sb[32 * h4 : 32 * h4 + D, gp, qsl],
                                in0=uo[64 * ii : 64 * ii + D, j, :],
                                in1=Rt[:, j, :],
                            )

                def emit_proj(qb):
                    # borrows a ps_s buffer (1 of its 3 banks); both m-halves
                    # run sequentially inside one allocation so the uo pool
                    # rotation stays strictly uo-only.
                    qsl = slice(QB * qb, QB * (qb + 1))
                    pp = ps_s.tile([128, 2, QB], F32, tag="s", name="psp")
                    for m in range(2):
                        for k in range(2):
                            nc.tensor.matmul(
                                pp[:, m, :],
                                wp_sb[:, k, 128 * m : 128 * (m + 1)],
                                ot_sb[:, k, qsl],
                                start=(k == 0),
                                stop=(k == 1),
                            )
                        po = po_pool.tile([128, QB], F32, tag="po", name="po")
                        nc.vector.tensor_scalar_add(
                            out=po, in0=pp[:, m, :], scalar1=bp_sb[:, m : m + 1]
                        )
                        nc.sync.dma_start(
                            out=out_d[128 * m : 128 * (m + 1), qsl], in_=po
                        )

                pending = None
                uo = None
                to_norm = None  # finished pair awaiting normalize
                to_proj = None  # qb awaiting projection
                for hidx, (qb, gp, hp) in enumerate(halves):
                    if hp == 0:
                        uo = ps_u.tile([128, 2, QB], F32, tag="u", name="psu")
                    pv_lo = 0
                    for g in range(NG):
                        ps, eslot = emit_scores_group(hidx, qb, gp, hp, g)
                        emit_exp(ps, eslot, GSIZES[g])
                        if pending is not None:
                            pv_hi = (2 * T) * (g + 1) // NG
                            emit_pv(pending, pv_lo, pv_hi)
                            pv_lo = pv_hi
                        # deferred retire work from the previous pair, placed
                        # mid-body so the engine FIFOs always have ready work
                        # queued ahead of the normalize/proj dependency chain
                        if g == 1 and to_norm is not None:
                            emit_normalize(to_norm)
                            if to_norm[1] == 1:  # gp == 1: query block done
                                to_proj = to_norm[0]
                            to_norm = None
                        if g == 6 and to_proj is not None:
                            emit_proj(to_proj)
                            to_proj = None
                    if pending is not None:
                        emit_pv(pending, pv_lo, 2 * T)
                        if pending[3] == 1:  # finished an (qb, gp) pair
                            to_norm = (pending[1], pending[2], pending[4])
                    pending = (hidx, qb, gp, hp, uo)
                # drain
                emit_pv(pending, 0, 2 * T)
                if to_norm is not None:
                    emit_normalize(to_norm)
                    if to_norm[1] == 1:
                        emit_proj(to_norm[0])
                    to_norm = None
                emit_normalize((pending[1], pending[2], pending[4]))
                emit_proj(pending[1])

    nc.compile()
    return nc


def _make_in_maps(x, w_qkv, b_qkv, w_proj, b_proj, n_cores=8):
    B, Cx, Hi, Wi = x.shape
    S = Hi * Wi
    NQ = S * B // n_cores
    xr = np.ascontiguousarray(x.reshape(B, Cx, S).astype(np.float32))
    wqkvT = np.ascontiguousarray(w_qkv.astype(np.float32).T)
    bq = np.ascontiguousarray(b_qkv[:Cx].astype(np.float32).reshape(Cx, 1))
    bv = b_qkv[2 * Cx : 3 * Cx].astype(np.float32)
    wpT = np.ascontiguousarray(w_proj.astype(np.float32).T.astype(np.float16))
    bp_eff = (b_proj.astype(np.float32) + w_proj.astype(np.float32) @ bv).reshape(
        Cx, 1
    )
    bp_eff = np.ascontiguousarray(bp_eff)
    halves = n_cores // B
    in_maps = []
    for core in range(n_cores):
        b, half = divmod(core, halves)
        xq = np.ascontiguousarray(xr[b][:, half * NQ : (half + 1) * NQ])
        in_maps.append(
            {"xc": xr[b], "xq": xq, "wqkvT": wqkvT, "bq": bq, "wpT": wpT, "bp": bp_eff}
        )
    return in_maps, (B, Cx, Hi, Wi, S, NQ)


_NC_CACHE = {}


def run(x, w_qkv, b_qkv, w_proj, b_proj, trace=False, **spmd_kwargs):
    from concourse.bass_utils import run_bass_kernel_spmd

    in_maps, (B, Cx, Hi, Wi, S, NQ) = _make_in_maps(x, w_qkv, b_qkv, w_proj, b_proj)
    key = (S, NQ)
    if key not in _NC_CACHE:
        _NC_CACHE[key] = build_nc(S, NQ)
    nc = _NC_CACHE[key]
    res = run_bass_kernel_spmd(
        nc, in_maps, core_ids=list(range(8)), trace=trace, **spmd_kwargs
    )
    outs = [r["out"] for r in res.results]
    halves = 8 // B
    full = np.empty((B, Cx, S), np.float32)
    for b in range(B):
        for half in range(halves):
            full[b, :, half * NQ : (half + 1) * NQ] = outs[halves * b + half]
    return full.reshape(B, Cx, Hi, Wi), res


def kernel(x, w_qkv, b_qkv, w_proj, b_proj):
    out, _ = run(x, w_qkv, b_qkv, w_proj, b_proj)
    return out
